# revision 21
# baseline (speedup 1.0000x reference)
"""BiMambaTextEncoder Trainium2 kernel.

Sharding: 8 cores = 4 batch x 2 direction. The backward direction is handled
by reversing the token sequence on the host and flipping the conv kernels, so
all cores run the same SPMD program. The final projection decomposes as
  concat([fo, bo]) @ proj_w.T = fo @ proj_w[:, :C].T + bo @ proj_w[:, C:].T
so each core computes its half and the host sums the pair (no collectives).

The whole network is processed chunk-major over two 512-column time chunks so
the tile scheduler can overlap chunk 1's conv stack (TensorE-bound) with chunk
0's selective-scan phase (VectorE-bound). Conv chunk 0 computes a few extra
halo columns per layer (mini psum tiles) so chunk 1 never needs left context
that doesn't exist yet.

Scan phase: for each (i-block, n-half) the 8 per-state scans are packed into
one [128, 8*513] tensor_tensor_scan; column n*513 is a boundary column with
dA=0 and dBu=carry-in state, which restarts the recurrence per state. All
elementwise scan-phase work stays on the Vector engine: the GpSimd engine
shares SBUF ports with it and running both concurrently halves both. B/C and
LN-row broadcasts are DMAd from DRAM bounce buffers (0-stride partition
reads). 1/sqrt(var) is computed as exp(-0.5*ln(var)) so the conv stack and
the scan phase share one ACT function table (natural_log_exp_and_others).
"""

from contextlib import ExitStack

import numpy as np

import concourse.bass as bass
from concourse import bacc
import concourse.mybir as mybir
import concourse.tile as tile
from concourse.bass_utils import run_bass_kernel_spmd

F16 = mybir.dt.float16
F32 = mybir.dt.float32
AF = mybir.ActivationFunctionType
OP = mybir.AluOpType

B, L, C, K, DEPTH, V = 4, 1024, 512, 5, 3, 178
VP = 192            # padded vocab (two K-tiles: 128 + 64)
DI = 1024           # d_inner
N = 16              # d_state
DCONV = 4
DTR = 32            # dt_rank
NCB = C // 128      # 4 channel blocks
NDB = DI // 128     # 8 d_inner blocks
TC = 2              # t chunks of 512
Q = 512
EPS = 1e-5

NH = 2              # n halves per block
NPH = N // NH       # 8 states per packed scan
WB = NPH * (Q + 1)  # 4104 packed scan width (513-stride blocks)

# conv output column ranges per (chunk, layer): chunk 0 overproduces the
# right halo each layer needs from its predecessor
CONV_RANGES = [
    [(0, Q + 4), (0, Q + 2), (0, Q)],
    [(Q + 4, L), (Q + 2, L), (Q, L)],
]


def _par(param, cob):
    s = param * NCB + cob
    return slice(s, s + 1)


def build_program():
    nc = bacc.Bacc()

    d_oh = nc.dram_tensor("oh", [VP, L], F16, kind="ExternalInput")
    d_embp = nc.dram_tensor("embp", [VP, C], F16, kind="ExternalInput")
    d_convw = nc.dram_tensor("convw", [DEPTH, NCB, 128, K * NCB, 128], F16,
                             kind="ExternalInput")
    d_cpar = nc.dram_tensor("cpar", [DEPTH, 128, 12], F32, kind="ExternalInput")
    d_inw = nc.dram_tensor("inw", [NCB, 128, 2 * DI], F16, kind="ExternalInput")
    d_mcw = nc.dram_tensor("mcw", [NDB, 128, DCONV * 128], F16,
                           kind="ExternalInput")
    d_mpar = nc.dram_tensor("mpar", [128, 16], F32, kind="ExternalInput")
    d_xw = nc.dram_tensor("xw", [NDB, 128, DTR + 2 * N], F16,
                          kind="ExternalInput")
    d_dtw = nc.dram_tensor("dtw", [NDB, DTR, 128], F16, kind="ExternalInput")
    d_An = nc.dram_tensor("An", [NDB, 128, N], F32, kind="ExternalInput")
    d_Dd = nc.dram_tensor("Dd", [NDB, 128, 128], F16, kind="ExternalInput")
    d_outw = nc.dram_tensor("outw", [NDB, 128, C], F16, kind="ExternalInput")
    d_pw = nc.dram_tensor("pw", [NCB, 128, C], F16, kind="ExternalInput")
    d_ident = nc.dram_tensor("ident", [128, 128], F16, kind="ExternalInput")
    d_part = nc.dram_tensor("part", [C, L], F32, kind="ExternalOutput")
    # DRAM bounce buffers for broadcast reads
    d_xbc = nc.dram_tensor("xbc", [2 * N, L], F16)
    d_lnr = [nc.dram_tensor(f"lnr{i}", [2, Q + 8], F16) for i in range(3)]

    with tile.TileContext(nc) as tc, ExitStack() as ctx:
        sing = ctx.enter_context(tc.tile_pool(name="sing", bufs=1))
        wp = ctx.enter_context(tc.tile_pool(name="wp", bufs=1))
        hp = ctx.enter_context(tc.tile_pool(name="hp", bufs=1))
        bcp = ctx.enter_context(tc.tile_pool(name="bcp", bufs=1))
        sp = ctx.enter_context(tc.tile_pool(name="sp", bufs=1))
        st = ctx.enter_context(tc.tile_pool(name="st", bufs=1))
        pp = ctx.enter_context(tc.tile_pool(name="pp", bufs=1, space="PSUM"))

        dma = nc.sync.dma_start

        def T(pool, shape, dt, tag, bufs, name):
            return pool.tile(shape, dt, tag=tag, bufs=bufs, name=name)

        def r3(t):
            return t[:].rearrange("p (n q) -> p n q", n=NPH)

        # ---- constants / params ----
        ident = T(sing, [128, 128], F16, "ident", 1, "ident")
        dma(out=ident[:], in_=d_ident[:])
        ones = T(sing, [128, 1], F16, "ones", 1, "ones")
        nc.vector.memset(ones[:], 1.0)
        ones32 = T(sing, [128, 1], F32, "ones32", 1, "ones32")
        nc.vector.memset(ones32[:], 1.0)
        epst = T(sing, [1, 1], F32, "epst", 1, "epst")
        nc.vector.memset(epst[:], EPS)
        zcol = T(sing, [128, NPH], F16, "zcol", 1, "zcol")
        nc.vector.memset(zcol[:], 0.0)
        cpar = []
        for l in range(DEPTH):
            t = T(sing, [128, 12], F32, f"cpar{l}", 1, f"cpar{l}")
            dma(out=t[:], in_=d_cpar[l])
            cpar.append(t)
        mpar = T(sing, [128, 16], F32, "mpar", 1, "mpar")
        dma(out=mpar[:], in_=d_mpar[:])
        An = []
        for i in range(NDB):
            t = T(sing, [128, N], F32, f"An{i}", 1, f"An{i}")
            dma(out=t[:], in_=d_An[i])
            An.append(t)
        states = []
        for i in range(NDB):
            t = T(sing, [128, N], F16, f"stt{i}", 1, f"stt{i}")
            states.append(t)

        # pre-touch every ACT-consumed param tile on the scalar engine so the
        # real consumers don't exceed the Activation ISA sync-wait limit (the
        # engine's vector clock subsumes the DMA deps after one wait)
        touch = T(sing, [128, 224], F32, "touch", 1, "touch")
        for ti_, tt_ in enumerate(cpar + [mpar] + An):
            w_ = tt_.shape[-1]
            nc.scalar.copy(out=touch[:, ti_ * 16: ti_ * 16 + w_], in_=tt_[:])
        nc.scalar.copy(out=touch[0:1, 223:224], in_=epst[:])

        # ---- small weights (loaded once) ----
        xw = []
        for i in range(NDB):
            t = T(wp, [128, DTR + 2 * N], F16, f"xw{i}", 1, f"xw{i}")
            dma(out=t[:], in_=d_xw[i])
            xw.append(t)
        dtw = []
        for i in range(NDB):
            t = T(wp, [DTR, 128], F16, f"dtw{i}", 1, f"dtw{i}")
            dma(out=t[:], in_=d_dtw[i])
            dtw.append(t)
        Dd = []
        for i in range(NDB):
            t = T(wp, [128, 128], F16, "Dd", 8, f"Dd{i}")
            dma(out=t[:], in_=d_Dd[i])
            Dd.append(t)

        # ---- embedding (one-hot matmul, full L up front) ----
        oh0 = T(hp, [128, L + 3], F16, "mid", 8, "oh0")
        oh1 = T(hp, [64, L], F16, "mid", 8, "oh1")
        dma(out=oh0[:, 0:L], in_=d_oh[0:128, :])
        dma(out=oh1[:], in_=d_oh[128:VP, :])
        emb0 = T(hp, [128, C], F16, "mid", 8, "emb0")
        emb1 = T(hp, [64, C], F16, "mid", 8, "emb1")
        dma(out=emb0[:], in_=d_embp[0:128, :])
        dma(out=emb1[:], in_=d_embp[128:VP, :])

        LP = L + 4
        hbuf = [[T(hp, [128, LP], F16, "big", 8, f"hbuf{s}_{cb}")
                 for cb in range(NCB)] for s in range(2)]
        for s in range(2):
            for cb in range(NCB):
                nc.vector.memset(hbuf[s][cb][:, 0:2], 0.0)
                nc.vector.memset(hbuf[s][cb][:, L + 2:LP], 0.0)

        for cb in range(NCB):
            for t in range(TC):
                ps = T(pp, [128, Q], F32, "mm", 2, f"ps_emb{cb}_{t}")
                nc.tensor.matmul(ps[:], emb0[:, cb * 128:(cb + 1) * 128],
                                 oh0[:, t * Q:(t + 1) * Q],
                                 start=True, stop=False)
                nc.tensor.matmul(ps[:], emb1[:, cb * 128:(cb + 1) * 128],
                                 oh1[:, t * Q:(t + 1) * Q],
                                 start=False, stop=True)
                nc.vector.tensor_copy(
                    out=hbuf[0][cb][:, 2 + t * Q: 2 + (t + 1) * Q], in_=ps[:])

        # full-L persistent activations
        LPAD = 3
        ubuf = [T(hp, [128, L + LPAD], F16, "mid", 8, f"ubuf{i}")
                for i in range(NDB)]
        for i in range(NDB):
            nc.vector.memset(ubuf[i][:, 0:LPAD], 0.0)
        silz = [T(hp, [128, L], F16, f"silz{i}", 1, f"silz{i}")
                for i in range(NDB)]
        uconv = [T(hp, [128, L], F16, f"uconv{i}", 1, f"uconv{i}")
                 for i in range(NDB)]
        xdbc = T(hp, [DTR + 2 * N, L], F16, "xdbc", 1, "xdbc")
        yfin = [T(hp, [128, L], F16, "big", 8, f"yfin{i}")
                for i in range(NDB)]

        # =========== chunk-major main loop ===========
        for tq in range(TC):
            sl = slice(tq * Q, (tq + 1) * Q)

            # ---- conv stack for this chunk ----
            for l in range(DEPTH):
                a, b = CONV_RANGES[tq][l]
                src = hbuf[l % 2]
                dst = hbuf[(l + 1) % 2]
                cw = []
                for cib in range(NCB):
                    t = T(wp, [128, K * NCB * 128], F16, "cw", 4,
                          f"cw{tq}_{l}_{cib}")
                    dma(out=t[:], in_=d_convw[l, cib])
                    cw.append(t)
                # sub-ranges of <=512 columns (main + optional halo mini)
                subs = []
                o = a
                while o < b:
                    w = min(Q, b - o)
                    subs.append((o, w))
                    o += w
                craw = []
                for cob in range(NCB):
                    cr = T(st, [128, Q + 4], F16, "craw", 4,
                           f"craw{tq}_{l}_{cob}")
                    for (o, w) in subs:
                        ps = T(pp, [128, Q], F32, "mm", 2,
                               f"ps_c{tq}_{l}_{cob}_{o}")
                        first = True
                        for cib in range(NCB):
                            for k in range(K):
                                j = k * NCB + cob
                                nc.tensor.matmul(
                                    ps[:, 0:w],
                                    cw[cib][:, j * 128:(j + 1) * 128],
                                    src[cib][:, o + k: o + k + w],
                                    start=first,
                                    stop=(cib == NCB - 1 and k == K - 1))
                                first = False
                        nc.scalar.activation(
                            out=cr[:, o - a:o - a + w], in_=ps[:, 0:w],
                            func=AF.Identity, bias=cpar[l][:, _par(0, cob)],
                            scale=1.0)
                    craw.append(cr)
                for (o, w) in subs:
                    co = o - a
                    ps_s = T(pp, [1, Q], F32, "st", 2, f"ps_s{tq}_{l}_{o}")
                    ps_q = T(pp, [1, Q], F32, "st", 2, f"ps_q{tq}_{l}_{o}")
                    for cob in range(NCB):
                        nc.tensor.matmul(ps_s[:, 0:w], ones[:],
                                         craw[cob][:, co:co + w],
                                         start=(cob == 0),
                                         stop=(cob == NCB - 1))
                    for cob in range(NCB):
                        sq = T(st, [128, Q], F16, "csq", 1,
                               f"csq{tq}_{l}_{cob}_{o}")
                        nc.scalar.activation(out=sq[:, 0:w],
                                             in_=craw[cob][:, co:co + w],
                                             func=AF.Square)
                        nc.tensor.matmul(ps_q[:, 0:w], ones[:], sq[:, 0:w],
                                         start=(cob == 0),
                                         stop=(cob == NCB - 1))
                    mu = T(st, [1, Q], F32, "row", 3, f"mu{tq}_{l}_{o}")
                    nc.vector.tensor_scalar_mul(mu[:, 0:w], ps_s[:, 0:w],
                                                1.0 / C)
                    var = T(st, [1, Q], F32, "row", 3, f"var{tq}_{l}_{o}")
                    # var = msq - mu^2
                    nc.vector.tensor_mul(var[:, 0:w], mu[:, 0:w], mu[:, 0:w])
                    nc.vector.tensor_scalar_mul(var[:, 0:w], var[:, 0:w],
                                                -1.0)
                    nc.vector.scalar_tensor_tensor(
                        out=var[:, 0:w], in0=ps_q[:, 0:w], scalar=1.0 / C,
                        in1=var[:, 0:w], op0=OP.mult, op1=OP.add)
                    # rstd = exp(-0.5*ln(var+eps)) -- keeps the conv stack on
                    # the same ACT table (ln/exp) as the scan phase
                    nc.scalar.activation(out=var[:, 0:w], in_=var[:, 0:w],
                                         func=AF.Ln, bias=epst[:], scale=1.0)
                    rstd = T(st, [1, Q], F32, "row", 3, f"rstd{tq}_{l}_{o}")
                    nc.scalar.activation(out=rstd[:, 0:w], in_=var[:, 0:w],
                                         func=AF.Exp, scale=-0.5)
                    nmr = T(st, [1, Q + 8], F16, "rowp", 1,
                            f"nmr{tq}_{l}_{o}")
                    nc.vector.tensor_mul(nmr[:, 0:w], mu[:, 0:w],
                                         rstd[:, 0:w])
                    nc.vector.tensor_scalar_mul(nmr[:, 0:w],
                                                nmr[:, 0:w], -1.0)
                    r16 = T(st, [1, Q + 8], F16, "rowp16", 1,
                            f"r16{tq}_{l}_{o}")
                    nc.vector.tensor_copy(out=r16[:, 0:w],
                                          in_=rstd[:, 0:w])
                    # broadcast the two LN rows via a DRAM bounce
                    dma(out=d_lnr[l][0:1, 0:w], in_=nmr[:, 0:w])
                    dma(out=d_lnr[l][1:2, 0:w], in_=r16[:, 0:w])
                    nm_bc = T(st, [128, Q], F16, "nm_bc", 2,
                              f"nm_bc{tq}_{l}_{o}")
                    dma(out=nm_bc[:, 0:w],
                        in_=d_lnr[l][0:1, 0:w].partition_broadcast(128))
                    rs_bc = T(st, [128, Q], F16, "rs_bc", 2,
                              f"rs_bc{tq}_{l}_{o}")
                    dma(out=rs_bc[:, 0:w],
                        in_=d_lnr[l][1:2, 0:w].partition_broadcast(128))
                    for cob in range(NCB):
                        t2 = T(st, [128, Q], F16, "lnt", 1,
                               f"lnt{tq}_{l}_{cob}_{o}")
                        nc.vector.tensor_mul(t2[:, 0:w],
                                             craw[cob][:, co:co + w],
                                             rs_bc[:, 0:w])
                        nc.vector.tensor_add(t2[:, 0:w], t2[:, 0:w],
                                             nm_bc[:, 0:w])
                        nc.scalar.activation(
                            out=dst[cob][:, 2 + o: 2 + o + w],
                            in_=t2[:, 0:w], func=AF.Prelu,
                            bias=cpar[l][:, _par(2, cob)],
                            scale=cpar[l][:, _par(1, cob)], alpha=0.2)

            hfin = hbuf[DEPTH % 2]

            # ---- in_proj for this chunk ----
            inw = []
            for cib in range(NCB):
                t = T(wp, [128, 2 * DI], F16, "cw", 4, f"inw{tq}_{cib}")
                dma(out=t[:], in_=d_inw[cib])
                inw.append(t)
            for m in range(2 * NDB):
                ps = T(pp, [128, Q], F32, "mm", 2, f"ps_in{tq}_{m}")
                for cib in range(NCB):
                    nc.tensor.matmul(
                        ps[:], inw[cib][:, m * 128:(m + 1) * 128],
                        hfin[cib][:, 2 + tq * Q: 2 + (tq + 1) * Q],
                        start=(cib == 0), stop=(cib == NCB - 1))
                if m < NDB:
                    nc.scalar.copy(
                        out=ubuf[m][:, LPAD + tq * Q: LPAD + (tq + 1) * Q],
                        in_=ps[:])
                else:
                    nc.scalar.activation(
                        out=silz[m - NDB][:, sl], in_=ps[:], func=AF.Silu)

            # ---- causal depthwise conv + SiLU ----
            for i in range(NDB):
                mcwt = T(wp, [128, DCONV * 128], F16, "mcw", 2,
                         f"mcw{tq}_{i}")
                dma(out=mcwt[:], in_=d_mcw[i])
                ps = T(pp, [128, Q], F32, "mm", 2, f"ps_mc{tq}_{i}")
                for k in range(DCONV):
                    nc.tensor.matmul(
                        ps[:], mcwt[:, k * 128:(k + 1) * 128],
                        ubuf[i][:, tq * Q + k: tq * Q + k + Q],
                        start=(k == 0), stop=(k == DCONV - 1))
                nc.scalar.activation(
                    out=uconv[i][:, sl], in_=ps[:],
                    func=AF.Silu, bias=mpar[:, i:i + 1], scale=1.0)

            # ---- x_proj ----
            ps = T(pp, [DTR + 2 * N, Q], F32, "mm", 2, f"ps_x{tq}")
            for i in range(NDB):
                nc.tensor.matmul(ps[:], xw[i][:], uconv[i][:, sl],
                                 start=(i == 0), stop=(i == NDB - 1))
            nc.scalar.copy(out=xdbc[:, sl], in_=ps[:])
            dma(out=d_xbc[:, sl], in_=xdbc[DTR:DTR + 2 * N, sl])

            # ---- scan phase ----
            # B/C broadcast tiles per half via DMA (0-stride partition reads)
            ball, call_ = [], []
            for h in range(NH):
                bt = T(bcp, [128, WB], F16, "ball", 2, f"ball{tq}_{h}")
                dma(out=r3(bt)[:, :, 1:Q + 1],
                    in_=d_xbc[NPH * h:NPH * h + NPH,
                              sl].partition_broadcast(128))
                ball.append(bt)
                ct = T(bcp, [128, WB], F16, "call", 2, f"call{tq}_{h}")
                dma(out=r3(ct)[:, :, 1:Q + 1],
                    in_=d_xbc[N + NPH * h:N + NPH * h + NPH,
                              sl].partition_broadcast(128))
                call_.append(ct)
            # dt_proj -> softplus (exp then ln-with-bias-1) -> delta
            deltas = []
            dus = []
            for i in range(NDB):
                ps = T(pp, [128, Q], F32, "dtm", 2, f"ps_dt{tq}_{i}")
                nc.tensor.matmul(ps[:], dtw[i][:], xdbc[0:DTR, sl],
                                 start=True, stop=True)
                dl = T(sp, [128, Q], F16, "delta", 8, f"delta{tq}_{i}")
                nc.scalar.activation(out=dl[:], in_=ps[:], func=AF.Exp,
                                     bias=mpar[:, 8 + i:9 + i], scale=1.0)
                deltas.append(dl)
            for i in range(NDB):
                nc.scalar.activation(out=deltas[i][:], in_=deltas[i][:],
                                     func=AF.Ln, bias=ones32[:], scale=1.0)
            pend = []
            for i in range(NDB):
                yp = T(pp, [128, Q], F32, "y", 2, f"yp{tq}_{i}")
                nc.tensor.matmul(yp[:], Dd[i][:], uconv[i][:, sl],
                                 start=True, stop=False)
                du = T(sp, [128, Q], F16, "du", 2, f"du{tq}_{i}")
                nc.vector.tensor_mul(du[:], deltas[i][:], uconv[i][:, sl])
                dus.append(du)
                # deferred yfin of the previous block so the Vector engine
                # never waits on the PE tail of iteration i-1
                if pend:
                    pi, pyp = pend.pop()
                    nc.vector.tensor_mul(yfin[pi][:, sl], pyp[:],
                                         silz[pi][:, sl])
                for h in range(NH):
                    da = T(sp, [128, WB], F16, "DA", 2, f"da{tq}_{i}_{h}")
                    da3 = r3(da)
                    nc.vector.tensor_copy(out=da3[:, :, 0:1], in_=zcol[:])
                    for nl in range(NPH):
                        n = h * NPH + nl
                        nc.scalar.activation(
                            out=da3[:, nl:nl + 1, 1:Q + 1],
                            in_=deltas[i][:], func=AF.Exp,
                            scale=An[i][:, n:n + 1])
                    dbu = T(sp, [128, WB], F16, "DBU", 1, f"dbu{tq}_{i}_{h}")
                    dbu3 = r3(dbu)
                    if tq == 0:
                        nc.vector.tensor_copy(out=dbu3[:, :, 0:1],
                                              in_=zcol[:])
                    else:
                        nc.vector.tensor_copy(
                            out=dbu3[:, :, 0:1],
                            in_=states[i][:, h * NPH:(h + 1) * NPH])
                    du_b = dus[i][:].unsqueeze(1).broadcast_to(
                        [128, NPH, Q])
                    nc.vector.tensor_tensor(out=dbu3[:, :, 1:Q + 1],
                                            in0=du_b,
                                            in1=r3(ball[h])[:, :, 1:Q + 1],
                                            op=OP.mult)
                    ht = T(sp, [128, WB], F16, "H", 2, f"h{tq}_{i}_{h}")
                    nc.vector.tensor_tensor_scan(ht[:], da[:], dbu[:], 0.0,
                                                 OP.mult, OP.add)
                    h3 = r3(ht)
                    if tq < TC - 1:
                        nc.vector.tensor_copy(
                            out=states[i][:, h * NPH:(h + 1) * NPH],
                            in_=h3[:, :, Q:Q + 1])
                    hc = T(sp, [128, NPH * Q], F16, "HC", 1,
                           f"hc{tq}_{i}_{h}")
                    hc3 = hc[:].rearrange("p (n q) -> p n q", n=NPH)
                    nc.vector.tensor_tensor(out=hc3, in0=h3[:, :, 1:Q + 1],
                                            in1=r3(call_[h])[:, :, 1:Q + 1],
                                            op=OP.mult)
                    for nl in range(NPH):
                        nc.tensor.matmul(yp[:], ident[:],
                                         hc[:, nl * Q:(nl + 1) * Q],
                                         start=False,
                                         stop=(h == NH - 1 and nl == NPH - 1))
                pend.append((i, yp))
            pi, pyp = pend.pop()
            nc.vector.tensor_mul(yfin[pi][:, sl], pyp[:], silz[pi][:, sl])

        # ---- out_proj ----
        outw = []
        for i in range(NDB):
            t = T(hp, [128, C], F16, f"silz{i}", 1, f"outw{i}")
            dma(out=t[:], in_=d_outw[i])
            outw.append(t)
        yo = [T(hp, [128, L], F16, "mid", 8, f"yo{cb}") for cb in range(NCB)]
        for cb in range(NCB):
            for t in range(TC):
                ps = T(pp, [128, Q], F32, "mm", 2, f"ps_o{cb}_{t}")
                for i in range(NDB):
                    nc.tensor.matmul(ps[:],
                                     outw[i][:, cb * 128:(cb + 1) * 128],
                                     yfin[i][:, t * Q:(t + 1) * Q],
                                     start=(i == 0), stop=(i == NDB - 1))
                nc.scalar.copy(out=yo[cb][:, t * Q:(t + 1) * Q], in_=ps[:])

        # ---- final proj half ----
        pw = []
        for cib in range(NCB):
            t = T(hp, [128, C], F16, f"uconv{cib}", 1, f"pw{cib}")
            dma(out=t[:], in_=d_pw[cib])
            pw.append(t)
        for cb in range(NCB):
            for t in range(TC):
                ps = T(pp, [128, Q], F32, "mm", 2, f"ps_p{cb}_{t}")
                for cib in range(NCB):
                    nc.tensor.matmul(ps[:],
                                     pw[cib][:, cb * 128:(cb + 1) * 128],
                                     yo[cib][:, t * Q:(t + 1) * Q],
                                     start=(cib == 0), stop=(cib == NCB - 1))
                ot = T(hp, [128, Q], F32, "osb", 1, f"osb{cb}_{t}")
                nc.scalar.copy(out=ot[:], in_=ps[:])
                dma(out=d_part[cb * 128:(cb + 1) * 128, t * Q:(t + 1) * Q],
                    in_=ot[:])

    nc.compile()
    return nc


_cache = {}


def _prep_core_inputs(inputs, core):
    b = core >> 1
    rev = (core & 1) == 1
    p = "b_" if rev else "f_"
    f16 = np.float16
    f32 = np.float32

    toks = np.asarray(inputs["x"][b]).astype(np.int64)
    if rev:
        toks = toks[::-1]
    oh = np.zeros((VP, L), f16)
    oh[toks, np.arange(L)] = 1.0

    key = ("wts", p)
    if key not in _cache:
        embp = np.zeros((VP, C), f16)
        embp[:V] = np.asarray(inputs["emb"]).astype(f16)

        cw = np.asarray(inputs["conv_w"]).astype(f32)  # [D, cout, cin, K]
        if rev:
            cw = cw[:, :, :, ::-1]
        convw = np.empty((DEPTH, NCB, 128, K * NCB, 128), f16)
        for l in range(DEPTH):
            for cib in range(NCB):
                for k in range(K):
                    for cob in range(NCB):
                        blk = cw[l, cob * 128:(cob + 1) * 128,
                                 cib * 128:(cib + 1) * 128, k]
                        convw[l, cib, :, k * NCB + cob, :] = blk.T.astype(f16)
        cpar = np.zeros((DEPTH, 128, 12), f32)
        for l in range(DEPTH):
            for cob in range(NCB):
                cs = slice(cob * 128, (cob + 1) * 128)
                cpar[l, :, 0 * NCB + cob] = inputs["conv_b"][l][cs]
                cpar[l, :, 1 * NCB + cob] = inputs["ln_g"][l][cs]
                cpar[l, :, 2 * NCB + cob] = inputs["ln_b"][l][cs]

        in_w = np.asarray(inputs[p + "in_w"]).astype(f32)  # [2*DI, C]
        inw = np.empty((NCB, 128, 2 * DI), f16)
        for cib in range(NCB):
            inw[cib] = in_w[:, cib * 128:(cib + 1) * 128].T.astype(f16)

        mconv = np.asarray(inputs[p + "conv_w"]).astype(f32)  # [DI, 4]
        mcw = np.zeros((NDB, 128, DCONV * 128), f16)
        dd = np.arange(128)
        for i in range(NDB):
            for k in range(DCONV):
                mcw[i, dd, k * 128 + dd] = mconv[i * 128:(i + 1) * 128, k]

        mpar = np.zeros((128, 16), f32)
        for i in range(NDB):
            mpar[:, i] = inputs[p + "conv_b"][i * 128:(i + 1) * 128]
            mpar[:, 8 + i] = inputs[p + "dt_b"][i * 128:(i + 1) * 128]

        x_w = np.asarray(inputs[p + "x_w"]).astype(f32)  # [64, DI]
        xw = np.empty((NDB, 128, DTR + 2 * N), f16)
        for i in range(NDB):
            xw[i] = x_w[:, i * 128:(i + 1) * 128].T.astype(f16)

        dt_w = np.asarray(inputs[p + "dt_w"]).astype(f32)  # [DI, DTR]
        dtw = np.empty((NDB, DTR, 128), f16)
        for i in range(NDB):
            dtw[i] = dt_w[i * 128:(i + 1) * 128, :].T.astype(f16)

        An = (-np.exp(np.asarray(inputs[p + "A_log"]).astype(f32))
              ).reshape(NDB, 128, N).astype(f32)

        Dv = np.asarray(inputs[p + "D"]).astype(f32)
        Dd = np.zeros((NDB, 128, 128), f16)
        for i in range(NDB):
            Dd[i, dd, dd] = Dv[i * 128:(i + 1) * 128]

        out_w = np.asarray(inputs[p + "out_w"]).astype(f32)  # [C, DI]
        outw = np.empty((NDB, 128, C), f16)
        for i in range(NDB):
            outw[i] = out_w[:, i * 128:(i + 1) * 128].T.astype(f16)

        proj_w = np.asarray(inputs["proj_w"]).astype(f32)  # [C, 2C]
        half = proj_w[:, C:] if rev else proj_w[:, :C]
        pw = np.empty((NCB, 128, C), f16)
        for cib in range(NCB):
            pw[cib] = half[:, cib * 128:(cib + 1) * 128].T.astype(f16)

        _cache[key] = dict(
            embp=embp, convw=convw, cpar=cpar, inw=inw, mcw=mcw, mpar=mpar,
            xw=xw, dtw=dtw, An=An, Dd=Dd, outw=outw, pw=pw,
            ident=np.eye(128, dtype=f16))
    m = dict(_cache[key])
    m["oh"] = oh
    return m


def kernel(**inputs):
    if "nc" not in _cache:
        _cache["nc"] = build_program()
    nc = _cache["nc"]
    # weights are cached per direction for repeat calls; invalidate so a new
    # inputs dict is always re-prepared
    for k in [k for k in _cache if k != "nc"]:
        del _cache[k]
    in_maps = [_prep_core_inputs(inputs, c) for c in range(8)]
    res = run_bass_kernel_spmd(nc, in_maps, list(range(8)))
    parts = [r["part"] for r in res.results]
    proj_b = np.asarray(inputs["proj_b"]).astype(np.float32)
    out = np.empty((B, L, C), np.float32)
    for b in range(B):
        # note: the reference concatenates bo still in reversed time order
        comb = parts[2 * b] + parts[2 * b + 1]
        out[b] = comb.T + proj_b[None, :]
    m = np.asarray(inputs["m"])
    out = np.where(m[:, :, None], 0.0, out).astype(np.float32)
    return out


# revision 22
# speedup vs baseline: 1.0383x; 1.0383x over previous
"""BiMambaTextEncoder Trainium2 kernel.

Sharding: 8 cores = 4 batch x 2 direction. The backward direction is handled
by reversing the token sequence on the host and flipping the conv kernels, so
all cores run the same SPMD program. The final projection decomposes as
  concat([fo, bo]) @ proj_w.T = fo @ proj_w[:, :C].T + bo @ proj_w[:, C:].T
so each core computes its half and the host sums the pair (no collectives).

The whole network is processed chunk-major over two 512-column time chunks so
the tile scheduler can overlap chunk 1's conv stack (TensorE-bound) with chunk
0's selective-scan phase (VectorE-bound). Conv chunk 0 computes a few extra
halo columns per layer (mini psum tiles) so chunk 1 never needs left context
that doesn't exist yet.

Scan phase: for each (i-block, n-half) the 8 per-state scans are packed into
one [128, 8*513] tensor_tensor_scan; column n*513 is a boundary column with
dA=0 and dBu=carry-in state, which restarts the recurrence per state. All
elementwise scan-phase work stays on the Vector engine: the GpSimd engine
shares SBUF ports with it and running both concurrently halves both. B/C and
LN-row broadcasts are DMAd from DRAM bounce buffers (0-stride partition
reads). 1/sqrt(var) is computed as exp(-0.5*ln(var)) so the conv stack and
the scan phase share one ACT function table (natural_log_exp_and_others).
"""

from contextlib import ExitStack

import numpy as np

import concourse.bass as bass
from concourse import bacc
import concourse.mybir as mybir
import concourse.tile as tile
from concourse.bass_utils import run_bass_kernel_spmd

F16 = mybir.dt.float16
F32 = mybir.dt.float32
AF = mybir.ActivationFunctionType
OP = mybir.AluOpType

B, L, C, K, DEPTH, V = 4, 1024, 512, 5, 3, 178
VP = 192            # padded vocab (two K-tiles: 128 + 64)
DI = 1024           # d_inner
N = 16              # d_state
DCONV = 4
DTR = 32            # dt_rank
NCB = C // 128      # 4 channel blocks
NDB = DI // 128     # 8 d_inner blocks
TC = 2              # t chunks of 512
Q = 512
EPS = 1e-5

NH = 2              # n halves per block
NPH = N // NH       # 8 states per packed scan
WB = NPH * (Q + 1)  # 4104 packed scan width (513-stride blocks)

# conv output column ranges per (chunk, layer): chunk 0 overproduces the
# right halo each layer needs from its predecessor
CONV_RANGES = [
    [(0, Q + 4), (0, Q + 2), (0, Q)],
    [(Q + 4, L), (Q + 2, L), (Q, L)],
]


def _par(param, cob):
    s = param * NCB + cob
    return slice(s, s + 1)


def build_program():
    nc = bacc.Bacc()

    d_oh = nc.dram_tensor("oh", [VP, L], F16, kind="ExternalInput")
    d_embp = nc.dram_tensor("embp", [VP, C], F16, kind="ExternalInput")
    d_convw = nc.dram_tensor("convw", [DEPTH, NCB, 128, K * NCB, 128], F16,
                             kind="ExternalInput")
    d_cpar = nc.dram_tensor("cpar", [DEPTH, 128, 12], F32, kind="ExternalInput")
    d_inw = nc.dram_tensor("inw", [NCB, 128, 2 * DI], F16, kind="ExternalInput")
    d_mcw = nc.dram_tensor("mcw", [NDB, 128, DCONV * 128], F16,
                           kind="ExternalInput")
    d_mpar = nc.dram_tensor("mpar", [128, 16], F32, kind="ExternalInput")
    d_xw = nc.dram_tensor("xw", [NDB, 128, DTR + 2 * N], F16,
                          kind="ExternalInput")
    d_dtw = nc.dram_tensor("dtw", [NDB, DTR, 128], F16, kind="ExternalInput")
    d_An = nc.dram_tensor("An", [NDB, 128, N], F32, kind="ExternalInput")
    d_Dd = nc.dram_tensor("Dd", [NDB, 128, 128], F16, kind="ExternalInput")
    d_outw = nc.dram_tensor("outw", [NDB, 128, C], F16, kind="ExternalInput")
    d_pw = nc.dram_tensor("pw", [NCB, 128, C], F16, kind="ExternalInput")
    d_ident = nc.dram_tensor("ident", [128, 128], F16, kind="ExternalInput")
    d_part = nc.dram_tensor("part", [C, L], F32, kind="ExternalOutput")
    # DRAM bounce buffers for broadcast reads
    d_xbc = nc.dram_tensor("xbc", [2 * N, L], F16)
    d_lnr = [nc.dram_tensor(f"lnr{i}", [2, Q + 8], F16) for i in range(3)]

    with tile.TileContext(nc) as tc, ExitStack() as ctx:
        sing = ctx.enter_context(tc.tile_pool(name="sing", bufs=1))
        wp = ctx.enter_context(tc.tile_pool(name="wp", bufs=1))
        hp = ctx.enter_context(tc.tile_pool(name="hp", bufs=1))
        bcp = ctx.enter_context(tc.tile_pool(name="bcp", bufs=1))
        sp = ctx.enter_context(tc.tile_pool(name="sp", bufs=1))
        st = ctx.enter_context(tc.tile_pool(name="st", bufs=1))
        pp = ctx.enter_context(tc.tile_pool(name="pp", bufs=1, space="PSUM"))

        dma = nc.sync.dma_start

        def T(pool, shape, dt, tag, bufs, name):
            return pool.tile(shape, dt, tag=tag, bufs=bufs, name=name)

        def r3(t):
            return t[:].rearrange("p (n q) -> p n q", n=NPH)

        # ---- constants / params ----
        ident = T(sing, [128, 128], F16, "ident", 1, "ident")
        dma(out=ident[:], in_=d_ident[:])
        ones = T(sing, [128, 1], F16, "ones", 1, "ones")
        nc.vector.memset(ones[:], 1.0)
        ones32 = T(sing, [128, 1], F32, "ones32", 1, "ones32")
        nc.vector.memset(ones32[:], 1.0)
        epst = T(sing, [1, 1], F32, "epst", 1, "epst")
        nc.vector.memset(epst[:], EPS)
        zcol = T(sing, [128, NPH], F16, "zcol", 1, "zcol")
        nc.vector.memset(zcol[:], 0.0)
        cpar = []
        for l in range(DEPTH):
            t = T(sing, [128, 12], F32, f"cpar{l}", 1, f"cpar{l}")
            dma(out=t[:], in_=d_cpar[l])
            cpar.append(t)
        mpar = T(sing, [128, 16], F32, "mpar", 1, "mpar")
        dma(out=mpar[:], in_=d_mpar[:])
        An = []
        for i in range(NDB):
            t = T(sing, [128, N], F32, f"An{i}", 1, f"An{i}")
            dma(out=t[:], in_=d_An[i])
            An.append(t)
        states = []
        for i in range(NDB):
            t = T(sing, [128, N], F16, f"stt{i}", 1, f"stt{i}")
            states.append(t)

        # pre-touch every ACT-consumed param tile on the scalar engine so the
        # real consumers don't exceed the Activation ISA sync-wait limit (the
        # engine's vector clock subsumes the DMA deps after one wait)
        touch = T(sing, [128, 224], F32, "touch", 1, "touch")
        for ti_, tt_ in enumerate(cpar + [mpar] + An):
            w_ = tt_.shape[-1]
            nc.scalar.copy(out=touch[:, ti_ * 16: ti_ * 16 + w_], in_=tt_[:])
        nc.scalar.copy(out=touch[0:1, 223:224], in_=epst[:])

        # ---- small weights (loaded once) ----
        xw = []
        for i in range(NDB):
            t = T(wp, [128, DTR + 2 * N], F16, f"xw{i}", 1, f"xw{i}")
            dma(out=t[:], in_=d_xw[i])
            xw.append(t)
        dtw = []
        for i in range(NDB):
            t = T(wp, [DTR, 128], F16, f"dtw{i}", 1, f"dtw{i}")
            dma(out=t[:], in_=d_dtw[i])
            dtw.append(t)
        Dd = []
        for i in range(NDB):
            t = T(wp, [128, 128], F16, "Dd", 8, f"Dd{i}")
            dma(out=t[:], in_=d_Dd[i])
            Dd.append(t)

        # ---- embedding (one-hot matmul, full L up front) ----
        oh0 = T(hp, [128, L + 3], F16, "mid", 8, "oh0")
        oh1 = T(hp, [64, L], F16, "mid", 8, "oh1")
        dma(out=oh0[:, 0:L], in_=d_oh[0:128, :])
        dma(out=oh1[:], in_=d_oh[128:VP, :])
        emb0 = T(hp, [128, C], F16, "mid", 8, "emb0")
        emb1 = T(hp, [64, C], F16, "mid", 8, "emb1")
        dma(out=emb0[:], in_=d_embp[0:128, :])
        dma(out=emb1[:], in_=d_embp[128:VP, :])

        LP = L + 4
        hbuf = [[T(hp, [128, LP], F16, "big", 8, f"hbuf{s}_{cb}")
                 for cb in range(NCB)] for s in range(2)]
        for s in range(2):
            for cb in range(NCB):
                nc.vector.memset(hbuf[s][cb][:, 0:2], 0.0)
                nc.vector.memset(hbuf[s][cb][:, L + 2:LP], 0.0)

        for cb in range(NCB):
            for t in range(TC):
                ps = T(pp, [128, Q], F32, "mm", 2, f"ps_emb{cb}_{t}")
                nc.tensor.matmul(ps[:], emb0[:, cb * 128:(cb + 1) * 128],
                                 oh0[:, t * Q:(t + 1) * Q],
                                 start=True, stop=False)
                nc.tensor.matmul(ps[:], emb1[:, cb * 128:(cb + 1) * 128],
                                 oh1[:, t * Q:(t + 1) * Q],
                                 start=False, stop=True)
                nc.vector.tensor_copy(
                    out=hbuf[0][cb][:, 2 + t * Q: 2 + (t + 1) * Q], in_=ps[:])

        # full-L persistent activations
        LPAD = 3
        ubuf = [T(hp, [128, L + LPAD], F16, "mid", 8, f"ubuf{i}")
                for i in range(NDB)]
        for i in range(NDB):
            nc.vector.memset(ubuf[i][:, 0:LPAD], 0.0)
        silz = [T(hp, [128, L], F16, f"silz{i}", 1, f"silz{i}")
                for i in range(NDB)]
        uconv = [T(hp, [128, L], F16, f"uconv{i}", 1, f"uconv{i}")
                 for i in range(NDB)]
        xdbc = T(hp, [DTR + 2 * N, L], F16, "xdbc", 1, "xdbc")
        yfin = [T(hp, [128, L], F16, "big", 8, f"yfin{i}")
                for i in range(NDB)]

        # =========== chunk-major main loop ===========
        def conv_layer(tq, l):
                a, b = CONV_RANGES[tq][l]
                src = hbuf[l % 2]
                dst = hbuf[(l + 1) % 2]
                cw = []
                for cib in range(NCB):
                    t = T(wp, [128, K * NCB * 128], F16, "cw", 4,
                          f"cw{tq}_{l}_{cib}")
                    dma(out=t[:], in_=d_convw[l, cib])
                    cw.append(t)
                # sub-ranges of <=512 columns (main + optional halo mini)
                subs = []
                o = a
                while o < b:
                    w = min(Q, b - o)
                    subs.append((o, w))
                    o += w
                craw = []
                for cob in range(NCB):
                    cr = T(st, [128, Q + 4], F16, "craw", 4,
                           f"craw{tq}_{l}_{cob}")
                    for (o, w) in subs:
                        ps = T(pp, [128, Q], F32, "mm", 2,
                               f"ps_c{tq}_{l}_{cob}_{o}")
                        first = True
                        for cib in range(NCB):
                            for k in range(K):
                                j = k * NCB + cob
                                nc.tensor.matmul(
                                    ps[:, 0:w],
                                    cw[cib][:, j * 128:(j + 1) * 128],
                                    src[cib][:, o + k: o + k + w],
                                    start=first,
                                    stop=(cib == NCB - 1 and k == K - 1))
                                first = False
                        nc.scalar.activation(
                            out=cr[:, o - a:o - a + w], in_=ps[:, 0:w],
                            func=AF.Identity, bias=cpar[l][:, _par(0, cob)],
                            scale=1.0)
                    craw.append(cr)
                for (o, w) in subs:
                    co = o - a
                    ps_s = T(pp, [1, Q], F32, "st", 2, f"ps_s{tq}_{l}_{o}")
                    ps_q = T(pp, [1, Q], F32, "st", 2, f"ps_q{tq}_{l}_{o}")
                    for cob in range(NCB):
                        nc.tensor.matmul(ps_s[:, 0:w], ones[:],
                                         craw[cob][:, co:co + w],
                                         start=(cob == 0),
                                         stop=(cob == NCB - 1))
                    for cob in range(NCB):
                        sq = T(st, [128, Q], F16, "csq", 1,
                               f"csq{tq}_{l}_{cob}_{o}")
                        nc.scalar.activation(out=sq[:, 0:w],
                                             in_=craw[cob][:, co:co + w],
                                             func=AF.Square)
                        nc.tensor.matmul(ps_q[:, 0:w], ones[:], sq[:, 0:w],
                                         start=(cob == 0),
                                         stop=(cob == NCB - 1))
                    mu = T(st, [1, Q], F32, "row", 3, f"mu{tq}_{l}_{o}")
                    nc.vector.tensor_scalar_mul(mu[:, 0:w], ps_s[:, 0:w],
                                                1.0 / C)
                    var = T(st, [1, Q], F32, "row", 3, f"var{tq}_{l}_{o}")
                    # var = msq - mu^2
                    nc.vector.tensor_mul(var[:, 0:w], mu[:, 0:w], mu[:, 0:w])
                    nc.vector.tensor_scalar_mul(var[:, 0:w], var[:, 0:w],
                                                -1.0)
                    nc.vector.scalar_tensor_tensor(
                        out=var[:, 0:w], in0=ps_q[:, 0:w], scalar=1.0 / C,
                        in1=var[:, 0:w], op0=OP.mult, op1=OP.add)
                    # rstd = exp(-0.5*ln(var+eps)) -- keeps the conv stack on
                    # the same ACT table (ln/exp) as the scan phase
                    nc.scalar.activation(out=var[:, 0:w], in_=var[:, 0:w],
                                         func=AF.Ln, bias=epst[:], scale=1.0)
                    rstd = T(st, [1, Q], F32, "row", 3, f"rstd{tq}_{l}_{o}")
                    nc.scalar.activation(out=rstd[:, 0:w], in_=var[:, 0:w],
                                         func=AF.Exp, scale=-0.5)
                    nmr = T(st, [1, Q + 8], F16, "rowp", 1,
                            f"nmr{tq}_{l}_{o}")
                    nc.vector.tensor_mul(nmr[:, 0:w], mu[:, 0:w],
                                         rstd[:, 0:w])
                    nc.vector.tensor_scalar_mul(nmr[:, 0:w],
                                                nmr[:, 0:w], -1.0)
                    r16 = T(st, [1, Q + 8], F16, "rowp16", 1,
                            f"r16{tq}_{l}_{o}")
                    nc.vector.tensor_copy(out=r16[:, 0:w],
                                          in_=rstd[:, 0:w])
                    # broadcast the two LN rows via a DRAM bounce
                    dma(out=d_lnr[l][0:1, 0:w], in_=nmr[:, 0:w])
                    dma(out=d_lnr[l][1:2, 0:w], in_=r16[:, 0:w])
                    nm_bc = T(st, [128, Q], F16, "nm_bc", 2,
                              f"nm_bc{tq}_{l}_{o}")
                    dma(out=nm_bc[:, 0:w],
                        in_=d_lnr[l][0:1, 0:w].partition_broadcast(128))
                    rs_bc = T(st, [128, Q], F16, "rs_bc", 2,
                              f"rs_bc{tq}_{l}_{o}")
                    dma(out=rs_bc[:, 0:w],
                        in_=d_lnr[l][1:2, 0:w].partition_broadcast(128))
                    for cob in range(NCB):
                        t2 = T(st, [128, Q], F16, "lnt", 1,
                               f"lnt{tq}_{l}_{cob}_{o}")
                        nc.vector.tensor_mul(t2[:, 0:w],
                                             craw[cob][:, co:co + w],
                                             rs_bc[:, 0:w])
                        nc.vector.tensor_add(t2[:, 0:w], t2[:, 0:w],
                                             nm_bc[:, 0:w])
                        nc.scalar.activation(
                            out=dst[cob][:, 2 + o: 2 + o + w],
                            in_=t2[:, 0:w], func=AF.Prelu,
                            bias=cpar[l][:, _par(2, cob)],
                            scale=cpar[l][:, _par(1, cob)], alpha=0.2)

        def in_proj_half(tq, half):
            sl = slice(tq * Q, (tq + 1) * Q)
            hfin = hbuf[DEPTH % 2]
            inw = []
            for cib in range(NCB):
                t = T(wp, [128, 2 * DI], F16, "cw", 4,
                      f"inw{tq}_{half}_{cib}")
                dma(out=t[:], in_=d_inw[cib])
                inw.append(t)
            for m in range(half * NDB, (half + 1) * NDB):
                ps = T(pp, [128, Q], F32, "mm", 2, f"ps_in{tq}_{m}")
                for cib in range(NCB):
                    nc.tensor.matmul(
                        ps[:], inw[cib][:, m * 128:(m + 1) * 128],
                        hfin[cib][:, 2 + tq * Q: 2 + (tq + 1) * Q],
                        start=(cib == 0), stop=(cib == NCB - 1))
                if m < NDB:
                    nc.scalar.copy(
                        out=ubuf[m][:, LPAD + tq * Q: LPAD + (tq + 1) * Q],
                        in_=ps[:])
                else:
                    nc.scalar.activation(
                        out=silz[m - NDB][:, sl], in_=ps[:], func=AF.Silu)

        def mamba_conv(tq):
            sl = slice(tq * Q, (tq + 1) * Q)
            for i in range(NDB):
                mcwt = T(wp, [128, DCONV * 128], F16, "mcw", 2,
                         f"mcw{tq}_{i}")
                dma(out=mcwt[:], in_=d_mcw[i])
                ps = T(pp, [128, Q], F32, "mm", 2, f"ps_mc{tq}_{i}")
                for k in range(DCONV):
                    nc.tensor.matmul(
                        ps[:], mcwt[:, k * 128:(k + 1) * 128],
                        ubuf[i][:, tq * Q + k: tq * Q + k + Q],
                        start=(k == 0), stop=(k == DCONV - 1))
                nc.scalar.activation(
                    out=uconv[i][:, sl], in_=ps[:],
                    func=AF.Silu, bias=mpar[:, i:i + 1], scale=1.0)

        def x_proj(tq):
            sl = slice(tq * Q, (tq + 1) * Q)
            ps = T(pp, [DTR + 2 * N, Q], F32, "mm", 2, f"ps_x{tq}")
            for i in range(NDB):
                nc.tensor.matmul(ps[:], xw[i][:], uconv[i][:, sl],
                                 start=(i == 0), stop=(i == NDB - 1))
            nc.scalar.copy(out=xdbc[:, sl], in_=ps[:])
            dma(out=d_xbc[:, sl], in_=xdbc[DTR:DTR + 2 * N, sl])

        def frontend_pieces(tq):
            return [
                lambda: conv_layer(tq, 0),
                lambda: conv_layer(tq, 1),
                lambda: conv_layer(tq, 2),
                lambda: in_proj_half(tq, 0),
                lambda: in_proj_half(tq, 1),
                lambda: mamba_conv(tq),
                lambda: x_proj(tq),
            ]

        # chunk 0 frontend emitted whole, up front
        for piece in frontend_pieces(0):
            piece()

        for tq in range(TC):
            sl = slice(tq * Q, (tq + 1) * Q)
            # next chunk's frontend, interleaved into this chunk's scan loop
            pieces = frontend_pieces(tq + 1) if tq + 1 < TC else []

            # ---- scan phase ----
            # B/C broadcast tiles per half via DMA (0-stride partition reads)
            ball, call_ = [], []
            for h in range(NH):
                bt = T(bcp, [128, WB], F16, "ball", 2, f"ball{tq}_{h}")
                dma(out=r3(bt)[:, :, 1:Q + 1],
                    in_=d_xbc[NPH * h:NPH * h + NPH,
                              sl].partition_broadcast(128))
                ball.append(bt)
                ct = T(bcp, [128, WB], F16, "call", 2, f"call{tq}_{h}")
                dma(out=r3(ct)[:, :, 1:Q + 1],
                    in_=d_xbc[N + NPH * h:N + NPH * h + NPH,
                              sl].partition_broadcast(128))
                call_.append(ct)
            # dt_proj -> softplus (exp then ln-with-bias-1) -> delta
            deltas = []
            dus = []
            for i in range(NDB):
                ps = T(pp, [128, Q], F32, "dtm", 2, f"ps_dt{tq}_{i}")
                nc.tensor.matmul(ps[:], dtw[i][:], xdbc[0:DTR, sl],
                                 start=True, stop=True)
                dl = T(sp, [128, Q], F16, "delta", 8, f"delta{tq}_{i}")
                nc.scalar.activation(out=dl[:], in_=ps[:], func=AF.Exp,
                                     bias=mpar[:, 8 + i:9 + i], scale=1.0)
                deltas.append(dl)
            for i in range(NDB):
                nc.scalar.activation(out=deltas[i][:], in_=deltas[i][:],
                                     func=AF.Ln, bias=ones32[:], scale=1.0)
            pend = []
            for i in range(NDB):
                if pieces:
                    pieces.pop(0)()
                yp = T(pp, [128, Q], F32, "y", 2, f"yp{tq}_{i}")
                nc.tensor.matmul(yp[:], Dd[i][:], uconv[i][:, sl],
                                 start=True, stop=False)
                du = T(sp, [128, Q], F16, "du", 2, f"du{tq}_{i}")
                nc.vector.tensor_mul(du[:], deltas[i][:], uconv[i][:, sl])
                dus.append(du)
                # deferred yfin of the previous block so the Vector engine
                # never waits on the PE tail of iteration i-1
                if pend:
                    pi, pyp = pend.pop()
                    nc.vector.tensor_mul(yfin[pi][:, sl], pyp[:],
                                         silz[pi][:, sl])
                for h in range(NH):
                    da = T(sp, [128, WB], F16, "DA", 2, f"da{tq}_{i}_{h}")
                    da3 = r3(da)
                    nc.vector.tensor_copy(out=da3[:, :, 0:1], in_=zcol[:])
                    for nl in range(NPH):
                        n = h * NPH + nl
                        nc.scalar.activation(
                            out=da3[:, nl:nl + 1, 1:Q + 1],
                            in_=deltas[i][:], func=AF.Exp,
                            scale=An[i][:, n:n + 1])
                    dbu = T(sp, [128, WB], F16, "DBU", 1, f"dbu{tq}_{i}_{h}")
                    dbu3 = r3(dbu)
                    if tq == 0:
                        nc.vector.tensor_copy(out=dbu3[:, :, 0:1],
                                              in_=zcol[:])
                    else:
                        nc.vector.tensor_copy(
                            out=dbu3[:, :, 0:1],
                            in_=states[i][:, h * NPH:(h + 1) * NPH])
                    du_b = dus[i][:].unsqueeze(1).broadcast_to(
                        [128, NPH, Q])
                    nc.vector.tensor_tensor(out=dbu3[:, :, 1:Q + 1],
                                            in0=du_b,
                                            in1=r3(ball[h])[:, :, 1:Q + 1],
                                            op=OP.mult)
                    ht = T(sp, [128, WB], F16, "H", 2, f"h{tq}_{i}_{h}")
                    nc.vector.tensor_tensor_scan(ht[:], da[:], dbu[:], 0.0,
                                                 OP.mult, OP.add)
                    h3 = r3(ht)
                    if tq < TC - 1:
                        nc.vector.tensor_copy(
                            out=states[i][:, h * NPH:(h + 1) * NPH],
                            in_=h3[:, :, Q:Q + 1])
                    hc = T(sp, [128, NPH * Q], F16, "HC", 1,
                           f"hc{tq}_{i}_{h}")
                    hc3 = hc[:].rearrange("p (n q) -> p n q", n=NPH)
                    nc.vector.tensor_tensor(out=hc3, in0=h3[:, :, 1:Q + 1],
                                            in1=r3(call_[h])[:, :, 1:Q + 1],
                                            op=OP.mult)
                    for nl in range(NPH):
                        nc.tensor.matmul(yp[:], ident[:],
                                         hc[:, nl * Q:(nl + 1) * Q],
                                         start=False,
                                         stop=(h == NH - 1 and nl == NPH - 1))
                pend.append((i, yp))
            pi, pyp = pend.pop()
            nc.vector.tensor_mul(yfin[pi][:, sl], pyp[:], silz[pi][:, sl])

        # ---- out_proj ----
        outw = []
        for i in range(NDB):
            t = T(hp, [128, C], F16, f"silz{i}", 1, f"outw{i}")
            dma(out=t[:], in_=d_outw[i])
            outw.append(t)
        yo = [T(hp, [128, L], F16, "mid", 8, f"yo{cb}") for cb in range(NCB)]
        for cb in range(NCB):
            for t in range(TC):
                ps = T(pp, [128, Q], F32, "mm", 2, f"ps_o{cb}_{t}")
                for i in range(NDB):
                    nc.tensor.matmul(ps[:],
                                     outw[i][:, cb * 128:(cb + 1) * 128],
                                     yfin[i][:, t * Q:(t + 1) * Q],
                                     start=(i == 0), stop=(i == NDB - 1))
                nc.scalar.copy(out=yo[cb][:, t * Q:(t + 1) * Q], in_=ps[:])

        # ---- final proj half ----
        pw = []
        for cib in range(NCB):
            t = T(hp, [128, C], F16, f"uconv{cib}", 1, f"pw{cib}")
            dma(out=t[:], in_=d_pw[cib])
            pw.append(t)
        for cb in range(NCB):
            for t in range(TC):
                ps = T(pp, [128, Q], F32, "mm", 2, f"ps_p{cb}_{t}")
                for cib in range(NCB):
                    nc.tensor.matmul(ps[:],
                                     pw[cib][:, cb * 128:(cb + 1) * 128],
                                     yo[cib][:, t * Q:(t + 1) * Q],
                                     start=(cib == 0), stop=(cib == NCB - 1))
                ot = T(hp, [128, Q], F32, "osb", 1, f"osb{cb}_{t}")
                nc.scalar.copy(out=ot[:], in_=ps[:])
                dma(out=d_part[cb * 128:(cb + 1) * 128, t * Q:(t + 1) * Q],
                    in_=ot[:])

    nc.compile()
    return nc


_cache = {}


def _prep_core_inputs(inputs, core):
    b = core >> 1
    rev = (core & 1) == 1
    p = "b_" if rev else "f_"
    f16 = np.float16
    f32 = np.float32

    toks = np.asarray(inputs["x"][b]).astype(np.int64)
    if rev:
        toks = toks[::-1]
    oh = np.zeros((VP, L), f16)
    oh[toks, np.arange(L)] = 1.0

    key = ("wts", p)
    if key not in _cache:
        embp = np.zeros((VP, C), f16)
        embp[:V] = np.asarray(inputs["emb"]).astype(f16)

        cw = np.asarray(inputs["conv_w"]).astype(f32)  # [D, cout, cin, K]
        if rev:
            cw = cw[:, :, :, ::-1]
        convw = np.empty((DEPTH, NCB, 128, K * NCB, 128), f16)
        for l in range(DEPTH):
            for cib in range(NCB):
                for k in range(K):
                    for cob in range(NCB):
                        blk = cw[l, cob * 128:(cob + 1) * 128,
                                 cib * 128:(cib + 1) * 128, k]
                        convw[l, cib, :, k * NCB + cob, :] = blk.T.astype(f16)
        cpar = np.zeros((DEPTH, 128, 12), f32)
        for l in range(DEPTH):
            for cob in range(NCB):
                cs = slice(cob * 128, (cob + 1) * 128)
                cpar[l, :, 0 * NCB + cob] = inputs["conv_b"][l][cs]
                cpar[l, :, 1 * NCB + cob] = inputs["ln_g"][l][cs]
                cpar[l, :, 2 * NCB + cob] = inputs["ln_b"][l][cs]

        in_w = np.asarray(inputs[p + "in_w"]).astype(f32)  # [2*DI, C]
        inw = np.empty((NCB, 128, 2 * DI), f16)
        for cib in range(NCB):
            inw[cib] = in_w[:, cib * 128:(cib + 1) * 128].T.astype(f16)

        mconv = np.asarray(inputs[p + "conv_w"]).astype(f32)  # [DI, 4]
        mcw = np.zeros((NDB, 128, DCONV * 128), f16)
        dd = np.arange(128)
        for i in range(NDB):
            for k in range(DCONV):
                mcw[i, dd, k * 128 + dd] = mconv[i * 128:(i + 1) * 128, k]

        mpar = np.zeros((128, 16), f32)
        for i in range(NDB):
            mpar[:, i] = inputs[p + "conv_b"][i * 128:(i + 1) * 128]
            mpar[:, 8 + i] = inputs[p + "dt_b"][i * 128:(i + 1) * 128]

        x_w = np.asarray(inputs[p + "x_w"]).astype(f32)  # [64, DI]
        xw = np.empty((NDB, 128, DTR + 2 * N), f16)
        for i in range(NDB):
            xw[i] = x_w[:, i * 128:(i + 1) * 128].T.astype(f16)

        dt_w = np.asarray(inputs[p + "dt_w"]).astype(f32)  # [DI, DTR]
        dtw = np.empty((NDB, DTR, 128), f16)
        for i in range(NDB):
            dtw[i] = dt_w[i * 128:(i + 1) * 128, :].T.astype(f16)

        An = (-np.exp(np.asarray(inputs[p + "A_log"]).astype(f32))
              ).reshape(NDB, 128, N).astype(f32)

        Dv = np.asarray(inputs[p + "D"]).astype(f32)
        Dd = np.zeros((NDB, 128, 128), f16)
        for i in range(NDB):
            Dd[i, dd, dd] = Dv[i * 128:(i + 1) * 128]

        out_w = np.asarray(inputs[p + "out_w"]).astype(f32)  # [C, DI]
        outw = np.empty((NDB, 128, C), f16)
        for i in range(NDB):
            outw[i] = out_w[:, i * 128:(i + 1) * 128].T.astype(f16)

        proj_w = np.asarray(inputs["proj_w"]).astype(f32)  # [C, 2C]
        half = proj_w[:, C:] if rev else proj_w[:, :C]
        pw = np.empty((NCB, 128, C), f16)
        for cib in range(NCB):
            pw[cib] = half[:, cib * 128:(cib + 1) * 128].T.astype(f16)

        _cache[key] = dict(
            embp=embp, convw=convw, cpar=cpar, inw=inw, mcw=mcw, mpar=mpar,
            xw=xw, dtw=dtw, An=An, Dd=Dd, outw=outw, pw=pw,
            ident=np.eye(128, dtype=f16))
    m = dict(_cache[key])
    m["oh"] = oh
    return m


def kernel(**inputs):
    if "nc" not in _cache:
        _cache["nc"] = build_program()
    nc = _cache["nc"]
    # weights are cached per direction for repeat calls; invalidate so a new
    # inputs dict is always re-prepared
    for k in [k for k in _cache if k != "nc"]:
        del _cache[k]
    in_maps = [_prep_core_inputs(inputs, c) for c in range(8)]
    res = run_bass_kernel_spmd(nc, in_maps, list(range(8)))
    parts = [r["part"] for r in res.results]
    proj_b = np.asarray(inputs["proj_b"]).astype(np.float32)
    out = np.empty((B, L, C), np.float32)
    for b in range(B):
        # note: the reference concatenates bo still in reversed time order
        comb = parts[2 * b] + parts[2 * b + 1]
        out[b] = comb.T + proj_b[None, :]
    m = np.asarray(inputs["m"])
    out = np.where(m[:, :, None], 0.0, out).astype(np.float32)
    return out
